# revision 1
# baseline (speedup 1.0000x reference)
"""AdaptiveCenterLoss on 8 TRN2 NeuronCores.

loss = sum((data - cen[labels])**2) / BATCH

Data-parallel over batch: each core handles 8192 rows, gathers its
center rows from a replicated `cen` table via indirect DMA, reduces to
a scalar partial sum on-device; host sums the 8 partials.
"""

import numpy as np

BATCH = 65536
DIM = 256
NUM_CLASSES = 100000
N_CORES = 8
B_CORE = BATCH // N_CORES  # 8192

P = 128          # SBUF partitions
K = 8            # batch rows per partition per tile
T = B_CORE // (P * K)  # tiles per core

_cached = {}


def _build_graph():
    from concourse import bass, bacc, mybir, tile

    nc = bacc.Bacc(
        "TRN2",
        target_bir_lowering=False,
        debug=False,
        num_devices=N_CORES,
    )
    f32 = mybir.dt.float32
    i32 = mybir.dt.int32

    data_t = nc.dram_tensor("data", [T, P, K * DIM], f32, kind="ExternalInput")
    lab_t = nc.dram_tensor("labels", [T, P, K], i32, kind="ExternalInput")
    cen_t = nc.dram_tensor("cen", [NUM_CLASSES, DIM], f32, kind="ExternalInput")
    out_t = nc.dram_tensor("out", [1, 1], f32, kind="ExternalOutput")

    with tile.TileContext(nc) as tc:
        with (
            tc.tile_pool(name="sbuf", bufs=3) as pool,
            tc.tile_pool(name="persist", bufs=1) as persist,
            tc.tile_pool(name="psum", bufs=1, space="PSUM") as psp,
        ):
            acc = persist.tile([P, 1], f32)
            ones = persist.tile([P, 1], f32)
            nc.gpsimd.memset(acc[:], 0.0)
            nc.gpsimd.memset(ones[:], 1.0)

            for t in range(T):
                lab = pool.tile([P, K], i32)
                nc.sync.dma_start(out=lab[:], in_=lab_t.ap()[t])

                ctr = pool.tile([P, K * DIM], f32)
                nc.gpsimd.indirect_dma_start(
                    out=ctr[:],
                    out_offset=None,
                    in_=cen_t.ap()[:],
                    in_offset=bass.IndirectOffsetOnAxis(ap=lab[:, :], axis=0),
                )

                dat = pool.tile([P, K * DIM], f32)
                nc.sync.dma_start(out=dat[:], in_=data_t.ap()[t])

                diff = pool.tile([P, K * DIM], f32)
                nc.vector.tensor_tensor(
                    out=diff[:], in0=dat[:], in1=ctr[:],
                    op=mybir.AluOpType.subtract,
                )

                sq = pool.tile([P, K * DIM], f32)
                part = pool.tile([P, 1], f32)
                nc.scalar.activation(
                    sq[:], diff[:],
                    mybir.ActivationFunctionType.Square,
                    accum_out=part[:],
                )
                nc.vector.tensor_tensor(
                    out=acc[:], in0=acc[:], in1=part[:],
                    op=mybir.AluOpType.add,
                )

            ps = psp.tile([1, 1], f32)
            nc.tensor.matmul(out=ps[:], lhsT=acc[:], rhs=ones[:], start=True, stop=True)
            res = persist.tile([1, 1], f32)
            nc.vector.tensor_copy(out=res[:], in_=ps[:])
            nc.sync.dma_start(out=out_t.ap()[:], in_=res[:])

    nc.compile()
    return nc


def _get_graph():
    if "nc" not in _cached:
        _cached["nc"] = _build_graph()
    return _cached["nc"]


def _make_in_maps(data, cen, labels):
    data = np.ascontiguousarray(np.asarray(data), dtype=np.float32)
    cen = np.ascontiguousarray(np.asarray(cen), dtype=np.float32)
    labels = np.ascontiguousarray(np.asarray(labels).astype(np.int32))
    in_maps = []
    for c in range(N_CORES):
        sl = slice(c * B_CORE, (c + 1) * B_CORE)
        in_maps.append(
            {
                "data": data[sl].reshape(T, P, K * DIM),
                "labels": labels[sl].reshape(T, P, K),
                "cen": cen,
            }
        )
    return in_maps


def _run(data, cen, labels, trace=False):
    from concourse.bass_utils import run_bass_kernel_spmd

    nc = _get_graph()
    in_maps = _make_in_maps(data, cen, labels)
    res = run_bass_kernel_spmd(nc, in_maps, core_ids=list(range(N_CORES)), trace=trace)
    total = sum(float(res.results[i]["out"][0, 0]) for i in range(N_CORES))
    return np.float32(total / BATCH), res


def kernel(data, cen, labels):
    out, _ = _run(data, cen, labels)
    return out
